# revision 3
# baseline (speedup 1.0000x reference)
"""CARAFE forward on 8 TRN2 NeuronCores.

Problem: features (8,128,64,64) f32, masks (8,25,128,128) f32
         -> out (8,128,128,128) f32, KERNEL_SIZE=5, GROUP=1, SCALE=2.

Sharding: pure data-parallel, one batch sample per core.

Formulation (banded matmul, i-pairs stacked along K):
  out[c, 2h+p, 2w+q] = sum_{i,j} f[c, h+i-2, w+j-2] * m[i*5+j, 2h+p, 2w+q]
For fixed (h, i) this is a matmul over x = w+j-2 (K=64):
  PSUM[c, col(p,w,q)] += sum_x f[c, r, x] * S(h,i)[x, col],  r = h+i-2
  S(h,i)[w+j-2, p*128+2w+q] = m[5i+j, 2h+p, 2w+q]  (banded; entries whose
  feature column is zero-padded are dropped).
Two consecutive i's share K=128 by stacking features of rows r and r+1.
Per h: 3 matmuls (i-pairs {0,1}, {2,3}, {4}), K=128, N=256, accumulated
into a 256-column slice of an 8-h PSUM mega-tile.

Performance structure (measured ~63 us/body/core, ~18.8 MB HBM traffic
at ~300 GB/s/core effective):
  - all arithmetic on the PE in fp16 (~3e-4 rel err vs fp32 reference);
  - device output is fp16, host casts to fp32 (halves output traffic,
    adds ~1e-4 rel err, both far under the 2e-2 gate);
  - input DMAs (ft2 2MB + two 6.3MB S chunks) stream on the SP HWDGE
    ring (nc.sync), output DMAs on the ACT ring (nc.scalar), double
    buffered; matmuls and DVE evacuation hide under the DMA stream.

_build_program(n_reps) with n_reps > 1 wraps the body in a hardware
For_i loop (program size constant in n_reps; ~0.7 us/iteration
back-edge) so replicated-execution timing contrasts measure the body,
not host-side program-size-proportional overheads.
"""

import numpy as np

N_CORES = 8
C, H, W = 128, 64, 64
K5 = 5
PAD = 2
KX = W                    # 64; K=128 after i-pair stacking
NCOL = 256                # (p, wo) output columns per low-res row h
NG = 3                    # i-groups per h: {0,1}, {2,3}, {4}
HB = 8                    # h rows per PSUM mega-tile
S_CHUNK = 32              # h rows per streamed S chunk
OB_ROWS = 16              # low-res h rows per output DMA

_compiled = {}


def _build_program(n_reps: int = 1):
    import concourse.bacc as bacc
    import concourse.mybir as mybir
    import concourse.tile as tile

    dt16 = mybir.dt.float16
    f32 = mybir.dt.float32
    nc = bacc.Bacc("TRN2", target_bir_lowering=False, debug=False,
                   num_devices=N_CORES)

    ft2 = nc.dram_tensor("ft2", [2 * KX, H, C], dt16, kind="ExternalInput")
    s2 = nc.dram_tensor("s2", [2 * KX, H, NG, NCOL], dt16,
                        kind="ExternalInput")
    out = nc.dram_tensor("out", [C, 2 * H, 2 * W], dt16,
                         kind="ExternalOutput")

    def h_matmuls(h):
        """(lhsT_h_index, k_size, group) triples for output row h."""
        mm = []
        if h == 1:
            mm.append((0, KX, 0))          # i=1 alone: lhsT top = f[:,0]
        elif h >= 2:
            mm.append((h - 2, 2 * KX, 0))  # i={0,1}: rows h-2, h-1
        mm.append((h, 2 * KX if h < H - 1 else KX, 1))  # i={2,3}
        if h + 2 < H:
            mm.append((h + 2, 2 * KX, 2))  # i=4: row h+2 (bottom half zero)
        return mm

    def body(sb, ss, ps, ob):
        ft_t = sb.tile([2 * KX, H, C], dt16, tag="ft")
        nc.scalar.dma_start(ft_t[:], ft2[:])
        for h0 in range(0, H, S_CHUNK):
            s_t = ss.tile([2 * KX, S_CHUNK, NG, NCOL], dt16, tag="s")
            seng = nc.scalar if h0 > 0 else nc.sync
            seng.dma_start(s_t[:], s2[:, h0:h0 + S_CHUNK, :, :])
            for p0 in range(h0, h0 + S_CHUNK, OB_ROWS):
                o = ob.tile([C, OB_ROWS * NCOL], dt16, tag="o")
                for b0 in range(p0, p0 + OB_ROWS, HB):
                    acc = ps.tile([C, HB * NCOL], f32)
                    for hl in range(HB):
                        h = b0 + hl
                        mms = h_matmuls(h)
                        o_sl = acc[:, hl * NCOL:(hl + 1) * NCOL]
                        for n_i, (hw, ks, g) in enumerate(mms):
                            nc.tensor.matmul(
                                o_sl, ft_t[0:ks, hw, :],
                                s_t[0:ks, h - h0, g, :],
                                start=(n_i == 0), stop=(n_i == len(mms) - 1))
                    off = (b0 - p0) * NCOL
                    nc.vector.tensor_copy(o[:, off:off + HB * NCOL], acc[:])
                nc.scalar.dma_start(
                    out[:, 2 * p0:2 * (p0 + OB_ROWS), :],
                    o[:].rearrange("c (hp w) -> c hp w", w=2 * W))

    with tile.TileContext(nc) as tc:
        with (
            tc.tile_pool(name="sb", bufs=1) as sb,
            tc.tile_pool(name="ss", bufs=2) as ss,
            tc.tile_pool(name="ps", bufs=2, space="PSUM") as ps,
            tc.tile_pool(name="ob", bufs=2) as ob,
        ):
            if n_reps == 1:
                body(sb, ss, ps, ob)
            else:
                with tc.For_i(0, n_reps, 1):
                    body(sb, ss, ps, ob)

    nc.compile()
    return nc


def _band(masks_n, i):
    """S(h,i) banded matrix for all h: [KX, H, 2, W, 2] from one sample's
    masks [25, 2H, 2W]; S[w+j-2, h, p, w, q] = m[5i+j, 2h+p, 2w+q]."""
    m = masks_n.reshape(K5, K5, H, 2, W, 2)  # [i, j, h, p, w, q]
    s = np.zeros((KX, H, 2, W, 2), dtype=np.float16)
    for j in range(K5):
        wlo = max(0, PAD - j)
        whi = min(W, W + PAD - j)
        wi = np.arange(wlo, whi)
        # dims (w, h, p, q) on both sides
        s[wi + j - PAD, :, :, wi, :] = m[i, j, :, :, wlo:whi].transpose(
            2, 0, 1, 3)
    return s


def _prep_inputs(features: np.ndarray, masks: np.ndarray):
    """Host-side layout prep (no FLOPs): stacked FT2 and grouped banded S2."""
    n = features.shape[0]
    ftw = features.transpose(0, 3, 2, 1).astype(np.float16)  # [n, w, h, c]
    ft2 = np.zeros((n, 2 * KX, H, C), dtype=np.float16)
    ft2[:, :KX] = ftw
    ft2[:, KX:, :H - 1] = ftw[:, :, 1:]      # row h+1; zero at h = H-1

    s2 = np.zeros((n, 2 * KX, H, NG, NCOL), dtype=np.float16)
    for smp in range(n):
        bands = [_band(masks[smp], i).reshape(KX, H, NCOL) for i in range(K5)]
        # group 0: i=0 (top, rows h-2 valid h>=2), i=1 (bottom, valid h>=1)
        s2[smp, :KX, 2:, 0] = bands[0][:, 2:]
        s2[smp, KX:, 2:, 0] = bands[1][:, 2:]
        s2[smp, :KX, 1, 0] = bands[1][:, 1]   # h=1 special: i=1 on top half
        # group 1: i=2 (top, always), i=3 (bottom, valid h <= H-2)
        s2[smp, :KX, :, 1] = bands[2]
        s2[smp, KX:, :H - 1, 1] = bands[3][:, :H - 1]
        # group 2: i=4 (top, valid h <= H-3); bottom stays zero
        s2[smp, :KX, :H - 2, 2] = bands[4][:, :H - 2]
    return ft2, s2


def kernel(features: np.ndarray, masks: np.ndarray) -> np.ndarray:
    from concourse.bass_utils import run_bass_kernel_spmd

    if 1 not in _compiled:
        _compiled[1] = _build_program(1)
    nc = _compiled[1]

    ft2, s2 = _prep_inputs(np.asarray(features, dtype=np.float32),
                           np.asarray(masks, dtype=np.float32))
    in_maps = [{"ft2": ft2[i], "s2": s2[i]} for i in range(N_CORES)]
    res = run_bass_kernel_spmd(nc, in_maps, list(range(N_CORES)))
    return np.stack(
        [res.results[i]["out"] for i in range(N_CORES)], axis=0
    ).astype(np.float32)


# revision 6
# speedup vs baseline: 1.0072x; 1.0072x over previous
"""CARAFE forward on 8 TRN2 NeuronCores.

Problem: features (8,128,64,64) f32, masks (8,25,128,128) f32
         -> out (8,128,128,128) f32, KERNEL_SIZE=5, GROUP=1, SCALE=2.

Sharding: pure data-parallel, one batch sample per core.

Formulation (banded matmul, i-pairs stacked along K):
  out[c, 2h+p, 2w+q] = sum_{i,j} f[c, h+i-2, w+j-2] * m[i*5+j, 2h+p, 2w+q]
For fixed (h, i) this is a matmul over x = w+j-2 (K=64):
  PSUM[c, col(p,w,q)] += sum_x f[c, r, x] * S(h,i)[x, col],  r = h+i-2
  S(h,i)[w+j-2, p*128+2w+q] = m[5i+j, 2h+p, 2w+q]  (banded; entries whose
  feature column is zero-padded are dropped).
Two consecutive i's share K=128 by stacking features of rows r and r+1.
Per h: 3 matmuls (i-pairs {0,1}, {2,3}, {4}), K=128, N=256, accumulated
into a 256-column slice of an 8-h PSUM mega-tile.

Performance structure (measured ~60 us/body/core, ~18.8 MB HBM traffic
at ~310 GB/s/core effective — at the HBM roofline):
  - all arithmetic on the PE in fp16 (~3e-4 rel err vs fp32 reference);
  - device output is fp16, host casts to fp32 (halves output traffic,
    adds ~1e-4 rel err, both far under the 2e-2 gate);
  - the S stream pipelines in four 3.15MB chunks alternating between
    the SP (nc.sync) and ACT (nc.scalar) HWDGE rings so the first
    matmul starts after ~10us and chunk DMAs overlap compute; ft2 and
    output DMAs ride the ACT ring; matmuls and DVE evacuation hide
    under the DMA stream.

_build_program(n_reps) with n_reps > 1 wraps the body in a hardware
For_i loop (program size constant in n_reps; ~0.7 us/iteration
back-edge) so replicated-execution timing contrasts measure the body,
not host-side program-size-proportional overheads.
"""

import numpy as np

N_CORES = 8
C, H, W = 128, 64, 64
K5 = 5
PAD = 2
KX = W                    # 64; K=128 after i-pair stacking
NCOL = 256                # (p, wo) output columns per low-res row h
NG = 3                    # i-groups per h: {0,1}, {2,3}, {4}
HB = 8                    # h rows per PSUM mega-tile
SCHED = (16, 16, 16, 16)  # h rows per streamed S chunk (alternating rings)
OB_ROWS = 16              # low-res h rows per output DMA

_compiled = {}


def _build_program(n_reps: int = 1):
    import concourse.bacc as bacc
    import concourse.mybir as mybir
    import concourse.tile as tile

    dt16 = mybir.dt.float16
    f32 = mybir.dt.float32
    nc = bacc.Bacc("TRN2", target_bir_lowering=False, debug=False,
                   num_devices=N_CORES)

    ft2 = nc.dram_tensor("ft2", [2 * KX, H, C], dt16, kind="ExternalInput")
    s2 = nc.dram_tensor("s2", [2 * KX, H, NG, NCOL], dt16,
                        kind="ExternalInput")
    out = nc.dram_tensor("out", [C, 2 * H, 2 * W], dt16,
                         kind="ExternalOutput")

    def h_matmuls(h):
        """(lhsT_h_index, k_size, group) triples for output row h."""
        mm = []
        if h == 1:
            mm.append((0, KX, 0))          # i=1 alone: lhsT top = f[:,0]
        elif h >= 2:
            mm.append((h - 2, 2 * KX, 0))  # i={0,1}: rows h-2, h-1
        mm.append((h, 2 * KX if h < H - 1 else KX, 1))  # i={2,3}
        if h + 2 < H:
            mm.append((h + 2, 2 * KX, 2))  # i=4: row h+2 (bottom half zero)
        return mm

    def body(sb, pools, ps, ob):
        ft_t = sb.tile([2 * KX, H, C], dt16, tag="ft")
        nc.scalar.dma_start(ft_t[:], ft2[:])
        h0 = 0
        for ci, rows in enumerate(SCHED):
            s_t = pools[ci].tile([2 * KX, rows, NG, NCOL], dt16,
                                 tag=f"s{ci}")
            seng = nc.scalar if ci % 2 else nc.sync
            seng.dma_start(s_t[:], s2[:, h0:h0 + rows, :, :])
            for p0 in range(h0, h0 + rows, OB_ROWS):
                orows = min(OB_ROWS, h0 + rows - p0)
                o = ob.tile([C, OB_ROWS * NCOL], dt16, tag="o")
                for b0 in range(p0, p0 + orows, HB):
                    acc = ps.tile([C, HB * NCOL], f32)
                    for hl in range(HB):
                        h = b0 + hl
                        mms = h_matmuls(h)
                        o_sl = acc[:, hl * NCOL:(hl + 1) * NCOL]
                        for n_i, (hw, ks, g) in enumerate(mms):
                            nc.tensor.matmul(
                                o_sl, ft_t[0:ks, hw, :],
                                s_t[0:ks, h - h0, g, :],
                                start=(n_i == 0), stop=(n_i == len(mms) - 1))
                    off = (b0 - p0) * NCOL
                    nc.vector.tensor_copy(o[:, off:off + HB * NCOL], acc[:])
                nc.scalar.dma_start(
                    out[:, 2 * p0:2 * (p0 + orows), :],
                    o[:, 0:orows * NCOL].rearrange(
                        "c (hp w) -> c hp w", w=2 * W))
            h0 += rows

    import contextlib

    with tile.TileContext(nc) as tc:
        with contextlib.ExitStack() as stk:
            sb = stk.enter_context(tc.tile_pool(name="sb", bufs=1))
            pools = [
                stk.enter_context(tc.tile_pool(name=f"ss{ci}", bufs=1))
                for ci in range(len(SCHED))
            ]
            ps = stk.enter_context(
                tc.tile_pool(name="ps", bufs=2, space="PSUM"))
            ob = stk.enter_context(tc.tile_pool(name="ob", bufs=2))
            if n_reps == 1:
                body(sb, pools, ps, ob)
            else:
                with tc.For_i(0, n_reps, 1):
                    body(sb, pools, ps, ob)

    nc.compile()
    return nc


def _band(masks_n, i):
    """S(h,i) banded matrix for all h: [KX, H, 2, W, 2] from one sample's
    masks [25, 2H, 2W]; S[w+j-2, h, p, w, q] = m[5i+j, 2h+p, 2w+q]."""
    m = masks_n.reshape(K5, K5, H, 2, W, 2)  # [i, j, h, p, w, q]
    s = np.zeros((KX, H, 2, W, 2), dtype=np.float16)
    for j in range(K5):
        wlo = max(0, PAD - j)
        whi = min(W, W + PAD - j)
        wi = np.arange(wlo, whi)
        # dims (w, h, p, q) on both sides
        s[wi + j - PAD, :, :, wi, :] = m[i, j, :, :, wlo:whi].transpose(
            2, 0, 1, 3)
    return s


def _prep_inputs(features: np.ndarray, masks: np.ndarray):
    """Host-side layout prep (no FLOPs): stacked FT2 and grouped banded S2."""
    n = features.shape[0]
    ftw = features.transpose(0, 3, 2, 1).astype(np.float16)  # [n, w, h, c]
    ft2 = np.zeros((n, 2 * KX, H, C), dtype=np.float16)
    ft2[:, :KX] = ftw
    ft2[:, KX:, :H - 1] = ftw[:, :, 1:]      # row h+1; zero at h = H-1

    s2 = np.zeros((n, 2 * KX, H, NG, NCOL), dtype=np.float16)
    for smp in range(n):
        bands = [_band(masks[smp], i).reshape(KX, H, NCOL) for i in range(K5)]
        # group 0: i=0 (top, rows h-2 valid h>=2), i=1 (bottom, valid h>=1)
        s2[smp, :KX, 2:, 0] = bands[0][:, 2:]
        s2[smp, KX:, 2:, 0] = bands[1][:, 2:]
        s2[smp, :KX, 1, 0] = bands[1][:, 1]   # h=1 special: i=1 on top half
        # group 1: i=2 (top, always), i=3 (bottom, valid h <= H-2)
        s2[smp, :KX, :, 1] = bands[2]
        s2[smp, KX:, :H - 1, 1] = bands[3][:, :H - 1]
        # group 2: i=4 (top, valid h <= H-3); bottom stays zero
        s2[smp, :KX, :H - 2, 2] = bands[4][:, :H - 2]
    return ft2, s2


def kernel(features: np.ndarray, masks: np.ndarray) -> np.ndarray:
    from concourse.bass_utils import run_bass_kernel_spmd

    if 1 not in _compiled:
        _compiled[1] = _build_program(1)
    nc = _compiled[1]

    ft2, s2 = _prep_inputs(np.asarray(features, dtype=np.float32),
                           np.asarray(masks, dtype=np.float32))
    in_maps = [{"ft2": ft2[i], "s2": s2[i]} for i in range(N_CORES)]
    res = run_bass_kernel_spmd(nc, in_maps, list(range(N_CORES)))
    return np.stack(
        [res.results[i]["out"] for i in range(N_CORES)], axis=0
    ).astype(np.float32)


# revision 7
# speedup vs baseline: 1.0539x; 1.0464x over previous
"""CARAFE forward on 8 TRN2 NeuronCores.

Problem: features (8,128,64,64) f32, masks (8,25,128,128) f32
         -> out (8,128,128,128) f32, KERNEL_SIZE=5, GROUP=1, SCALE=2.

Sharding: pure data-parallel, one batch sample per core.

Formulation (banded matmul, i-pairs stacked along K):
  out[c, 2h+p, 2w+q] = sum_{i,j} f[c, h+i-2, w+j-2] * m[i*5+j, 2h+p, 2w+q]
For fixed (h, i) this is a matmul over x = w+j-2 (K=64):
  PSUM[c, col(p,w,q)] += sum_x f[c, r, x] * S(h,i)[x, col],  r = h+i-2
  S(h,i)[w+j-2, p*128+2w+q] = m[5i+j, 2h+p, 2w+q]  (banded; entries whose
  feature column is zero-padded are dropped).
Two consecutive i's share K=128 by stacking features of rows r and r+1.
Per h: 3 matmuls (i-pairs {0,1}, {2,3}, {4}), K=128, N=256, accumulated
into a 256-column slice of an 8-h PSUM mega-tile.

Performance structure (measured ~60 us/body/core, ~18.8 MB HBM traffic
at ~310 GB/s/core effective — at the HBM roofline):
  - all arithmetic on the PE in fp16 (~3e-4 rel err vs fp32 reference);
  - device output is fp16, host casts to fp32 (halves output traffic,
    adds ~1e-4 rel err, both far under the 2e-2 gate);
  - the S stream pipelines in four 3.15MB chunks alternating between
    the SP (nc.sync) and ACT (nc.scalar) HWDGE rings so the first
    matmul starts after ~10us and chunk DMAs overlap compute; ft2 and
    output DMAs ride the ACT ring; matmuls and DVE evacuation hide
    under the DMA stream.

_build_program(n_reps) with n_reps > 1 wraps the body in a hardware
For_i loop (program size constant in n_reps; ~0.7 us/iteration
back-edge) so replicated-execution timing contrasts measure the body,
not host-side program-size-proportional overheads.
"""

import numpy as np

N_CORES = 8
C, H, W = 128, 64, 64
K5 = 5
PAD = 2
KX = W                    # 64; K=128 after i-pair stacking
NCOL = 256                # (p, wo) output columns per low-res row h
NG = 3                    # i-groups per h: {0,1}, {2,3}, {4}
HB = 4                    # h rows per PSUM tile (2 banks)
SCHED = (16, 16, 16, 16)  # h rows per streamed S chunk (alternating rings)
OB_ROWS = 16              # low-res h rows per output DMA

_compiled = {}


def _build_program(n_reps: int = 1):
    import concourse.bacc as bacc
    import concourse.mybir as mybir
    import concourse.tile as tile

    dt16 = mybir.dt.float16
    f32 = mybir.dt.float32
    nc = bacc.Bacc("TRN2", target_bir_lowering=False, debug=False,
                   num_devices=N_CORES)

    ft2 = nc.dram_tensor("ft2", [2 * KX, H, C], dt16, kind="ExternalInput")
    s2 = nc.dram_tensor("s2", [2 * KX, H, NG, NCOL], dt16,
                        kind="ExternalInput")
    out = nc.dram_tensor("out", [C, 2 * H, 2 * W], dt16,
                         kind="ExternalOutput")

    def h_matmuls(h):
        """(lhsT_h_index, k_size, group) triples for output row h."""
        mm = []
        if h == 1:
            mm.append((0, KX, 0))          # i=1 alone: lhsT top = f[:,0]
        elif h >= 2:
            mm.append((h - 2, 2 * KX, 0))  # i={0,1}: rows h-2, h-1
        mm.append((h, 2 * KX if h < H - 1 else KX, 1))  # i={2,3}
        if h + 2 < H:
            mm.append((h + 2, 2 * KX, 2))  # i=4: row h+2 (bottom half zero)
        return mm

    def body(sb, pools, ps, ob):
        ft_t = sb.tile([2 * KX, H, C], dt16, tag="ft")
        nc.scalar.dma_start(ft_t[:], ft2[:])
        h0 = 0
        for ci, rows in enumerate(SCHED):
            s_t = pools[ci].tile([2 * KX, rows, NG, NCOL], dt16,
                                 tag=f"s{ci}")
            seng = nc.scalar if ci % 2 else nc.sync
            seng.dma_start(s_t[:], s2[:, h0:h0 + rows, :, :])
            for p0 in range(h0, h0 + rows, OB_ROWS):
                orows = min(OB_ROWS, h0 + rows - p0)
                o = ob.tile([C, OB_ROWS * NCOL], dt16, tag="o")
                for b0 in range(p0, p0 + orows, HB):
                    acc = ps.tile([C, HB * NCOL], f32)
                    for hl in range(HB):
                        h = b0 + hl
                        mms = h_matmuls(h)
                        o_sl = acc[:, hl * NCOL:(hl + 1) * NCOL]
                        for n_i, (hw, ks, g) in enumerate(mms):
                            nc.tensor.matmul(
                                o_sl, ft_t[0:ks, hw, :],
                                s_t[0:ks, h - h0, g, :],
                                start=(n_i == 0), stop=(n_i == len(mms) - 1))
                    off = (b0 - p0) * NCOL
                    nc.vector.tensor_copy(o[:, off:off + HB * NCOL], acc[:])
                nc.scalar.dma_start(
                    out[:, 2 * p0:2 * (p0 + orows), :],
                    o[:, 0:orows * NCOL].rearrange(
                        "c (hp w) -> c hp w", w=2 * W))
            h0 += rows

    import contextlib

    with tile.TileContext(nc) as tc:
        with contextlib.ExitStack() as stk:
            sb = stk.enter_context(tc.tile_pool(name="sb", bufs=1))
            pools = [
                stk.enter_context(tc.tile_pool(name=f"ss{ci}", bufs=1))
                for ci in range(len(SCHED))
            ]
            ps = stk.enter_context(
                tc.tile_pool(name="ps", bufs=4, space="PSUM"))
            ob = stk.enter_context(tc.tile_pool(name="ob", bufs=3))
            if n_reps == 1:
                body(sb, pools, ps, ob)
            else:
                with tc.For_i(0, n_reps, 1):
                    body(sb, pools, ps, ob)

    nc.compile()
    return nc


def _band(masks_n, i):
    """S(h,i) banded matrix for all h: [KX, H, 2, W, 2] from one sample's
    masks [25, 2H, 2W]; S[w+j-2, h, p, w, q] = m[5i+j, 2h+p, 2w+q]."""
    m = masks_n.reshape(K5, K5, H, 2, W, 2)  # [i, j, h, p, w, q]
    s = np.zeros((KX, H, 2, W, 2), dtype=np.float16)
    for j in range(K5):
        wlo = max(0, PAD - j)
        whi = min(W, W + PAD - j)
        wi = np.arange(wlo, whi)
        # dims (w, h, p, q) on both sides
        s[wi + j - PAD, :, :, wi, :] = m[i, j, :, :, wlo:whi].transpose(
            2, 0, 1, 3)
    return s


def _prep_inputs(features: np.ndarray, masks: np.ndarray):
    """Host-side layout prep (no FLOPs): stacked FT2 and grouped banded S2."""
    n = features.shape[0]
    ftw = features.transpose(0, 3, 2, 1).astype(np.float16)  # [n, w, h, c]
    ft2 = np.zeros((n, 2 * KX, H, C), dtype=np.float16)
    ft2[:, :KX] = ftw
    ft2[:, KX:, :H - 1] = ftw[:, :, 1:]      # row h+1; zero at h = H-1

    s2 = np.zeros((n, 2 * KX, H, NG, NCOL), dtype=np.float16)
    for smp in range(n):
        bands = [_band(masks[smp], i).reshape(KX, H, NCOL) for i in range(K5)]
        # group 0: i=0 (top, rows h-2 valid h>=2), i=1 (bottom, valid h>=1)
        s2[smp, :KX, 2:, 0] = bands[0][:, 2:]
        s2[smp, KX:, 2:, 0] = bands[1][:, 2:]
        s2[smp, :KX, 1, 0] = bands[1][:, 1]   # h=1 special: i=1 on top half
        # group 1: i=2 (top, always), i=3 (bottom, valid h <= H-2)
        s2[smp, :KX, :, 1] = bands[2]
        s2[smp, KX:, :H - 1, 1] = bands[3][:, :H - 1]
        # group 2: i=4 (top, valid h <= H-3); bottom stays zero
        s2[smp, :KX, :H - 2, 2] = bands[4][:, :H - 2]
    return ft2, s2


def kernel(features: np.ndarray, masks: np.ndarray) -> np.ndarray:
    from concourse.bass_utils import run_bass_kernel_spmd

    if 1 not in _compiled:
        _compiled[1] = _build_program(1)
    nc = _compiled[1]

    ft2, s2 = _prep_inputs(np.asarray(features, dtype=np.float32),
                           np.asarray(masks, dtype=np.float32))
    in_maps = [{"ft2": ft2[i], "s2": s2[i]} for i in range(N_CORES)]
    res = run_bass_kernel_spmd(nc, in_maps, list(range(N_CORES)))
    return np.stack(
        [res.results[i]["out"] for i in range(N_CORES)], axis=0
    ).astype(np.float32)


# revision 8
# speedup vs baseline: 1.0655x; 1.0109x over previous
"""CARAFE forward on 8 TRN2 NeuronCores.

Problem: features (8,128,64,64) f32, masks (8,25,128,128) f32
         -> out (8,128,128,128) f32, KERNEL_SIZE=5, GROUP=1, SCALE=2.

Sharding: pure data-parallel, one batch sample per core.

Formulation (banded matmul, i-pairs stacked along K):
  out[c, 2h+p, 2w+q] = sum_{i,j} f[c, h+i-2, w+j-2] * m[i*5+j, 2h+p, 2w+q]
For fixed (h, i) this is a matmul over x = w+j-2 (K=64):
  PSUM[c, col(p,w,q)] += sum_x f[c, r, x] * S(h,i)[x, col],  r = h+i-2
  S(h,i)[w+j-2, p*128+2w+q] = m[5i+j, 2h+p, 2w+q]  (banded; entries whose
  feature column is zero-padded are dropped).
Two consecutive i's share K=128 by stacking features of rows r and r+1.
Per h: 3 matmuls (i-pairs {0,1}, {2,3}, {4}), K=128, N=256, accumulated
into a 256-column slice of an 8-h PSUM mega-tile.

Performance structure (measured ~60 us/body/core, ~18.8 MB HBM traffic
at ~310 GB/s/core effective — at the HBM roofline):
  - all arithmetic on the PE in fp16 (~3e-4 rel err vs fp32 reference);
  - device output is fp16, host casts to fp32 (halves output traffic,
    adds ~1e-4 rel err, both far under the 2e-2 gate);
  - the S stream pipelines in four 3.15MB chunks alternating between
    the SP (nc.sync) and ACT (nc.scalar) HWDGE rings so the first
    matmul starts after ~10us and chunk DMAs overlap compute; ft2 and
    output DMAs ride the ACT ring; matmuls and DVE evacuation hide
    under the DMA stream.

_build_program(n_reps) with n_reps > 1 wraps the body in a hardware
For_i loop (program size constant in n_reps; ~0.7 us/iteration
back-edge) so replicated-execution timing contrasts measure the body,
not host-side program-size-proportional overheads.
"""

import numpy as np

N_CORES = 8
C, H, W = 128, 64, 64
K5 = 5
PAD = 2
KX = W                    # 64; K=128 after i-pair stacking
NCOL = 256                # (p, wo) output columns per low-res row h
NG = 3                    # i-groups per h: {0,1}, {2,3}, {4}
HB = 4                    # h rows per PSUM tile (2 banks)
SCHED = (16, 16, 16, 16)  # h rows per streamed S chunk (alternating rings)
OB_ROWS = 16              # low-res h rows per output DMA

_compiled = {}


def _build_program(n_reps: int = 1):
    import concourse.bacc as bacc
    import concourse.mybir as mybir
    import concourse.tile as tile

    dt16 = mybir.dt.float16
    f32 = mybir.dt.float32
    nc = bacc.Bacc("TRN2", target_bir_lowering=False, debug=False,
                   num_devices=N_CORES)

    ft2 = nc.dram_tensor("ft2", [2 * KX, H, C], dt16, kind="ExternalInput")
    s2 = nc.dram_tensor("s2", [2 * KX, H, NG, NCOL], dt16,
                        kind="ExternalInput")
    out = nc.dram_tensor("out", [C, 2 * H, 2 * W], dt16,
                         kind="ExternalOutput")

    def h_matmuls(h):
        """(lhsT_h_index, k_size, group) triples for output row h."""
        mm = []
        if h == 1:
            mm.append((0, KX, 0))          # i=1 alone: lhsT top = f[:,0]
        elif h >= 2:
            mm.append((h - 2, 2 * KX, 0))  # i={0,1}: rows h-2, h-1
        mm.append((h, 2 * KX if h < H - 1 else KX, 1))  # i={2,3}
        if h + 2 < H:
            mm.append((h + 2, 2 * KX, 2))  # i=4: row h+2 (bottom half zero)
        return mm

    def body(sb, pools, ps, ob):
        ft_t = sb.tile([2 * KX, H, C], dt16, tag="ft")
        nc.scalar.dma_start(ft_t[:], ft2[:])
        h0 = 0
        for ci, rows in enumerate(SCHED):
            s_t = pools[ci].tile([2 * KX, rows, NG, NCOL], dt16,
                                 tag=f"s{ci}")
            seng = nc.scalar if ci % 2 else nc.sync
            seng.dma_start(s_t[:], s2[:, h0:h0 + rows, :, :])
            for p0 in range(h0, h0 + rows, OB_ROWS):
                orows = min(OB_ROWS, h0 + rows - p0)
                o = ob.tile([C, OB_ROWS * NCOL], dt16, tag="o")
                for b0 in range(p0, p0 + orows, HB):
                    acc = ps.tile([C, HB * NCOL], f32)
                    for hl in range(HB):
                        h = b0 + hl
                        mms = h_matmuls(h)
                        o_sl = acc[:, hl * NCOL:(hl + 1) * NCOL]
                        for n_i, (hw, ks, g) in enumerate(mms):
                            nc.tensor.matmul(
                                o_sl, ft_t[0:ks, hw, :],
                                s_t[0:ks, h - h0, g, :],
                                start=(n_i == 0), stop=(n_i == len(mms) - 1))
                    off = (b0 - p0) * NCOL
                    nc.vector.tensor_copy(o[:, off:off + HB * NCOL], acc[:])
                nc.scalar.dma_start(
                    out[:, 2 * p0:2 * (p0 + orows), :],
                    o[:, 0:orows * NCOL].rearrange(
                        "c (hp w) -> c hp w", w=2 * W))
            h0 += rows

    import contextlib

    with tile.TileContext(nc) as tc:
        with contextlib.ExitStack() as stk:
            sb = stk.enter_context(tc.tile_pool(name="sb", bufs=1))
            pools = [
                stk.enter_context(tc.tile_pool(name=f"ss{ci}", bufs=1))
                for ci in range(len(SCHED))
            ]
            ps = stk.enter_context(
                tc.tile_pool(name="ps", bufs=4, space="PSUM"))
            ob = stk.enter_context(tc.tile_pool(name="ob", bufs=3))
            if n_reps == 1:
                body(sb, pools, ps, ob)
            else:
                # PE body exceeds one 256-inst IRAM block; arm the branch
                # prefetcher so the back-edge I$-hits.
                with tc.For_i(0, n_reps, 1,
                              hint_engines=(mybir.EngineType.PE,)):
                    body(sb, pools, ps, ob)

    nc.compile()
    return nc


def _band(masks_n, i):
    """S(h,i) banded matrix for all h: [KX, H, 2, W, 2] from one sample's
    masks [25, 2H, 2W]; S[w+j-2, h, p, w, q] = m[5i+j, 2h+p, 2w+q]."""
    m = masks_n.reshape(K5, K5, H, 2, W, 2)  # [i, j, h, p, w, q]
    s = np.zeros((KX, H, 2, W, 2), dtype=np.float16)
    for j in range(K5):
        wlo = max(0, PAD - j)
        whi = min(W, W + PAD - j)
        wi = np.arange(wlo, whi)
        # dims (w, h, p, q) on both sides
        s[wi + j - PAD, :, :, wi, :] = m[i, j, :, :, wlo:whi].transpose(
            2, 0, 1, 3)
    return s


def _prep_inputs(features: np.ndarray, masks: np.ndarray):
    """Host-side layout prep (no FLOPs): stacked FT2 and grouped banded S2."""
    n = features.shape[0]
    ftw = features.transpose(0, 3, 2, 1).astype(np.float16)  # [n, w, h, c]
    ft2 = np.zeros((n, 2 * KX, H, C), dtype=np.float16)
    ft2[:, :KX] = ftw
    ft2[:, KX:, :H - 1] = ftw[:, :, 1:]      # row h+1; zero at h = H-1

    s2 = np.zeros((n, 2 * KX, H, NG, NCOL), dtype=np.float16)
    for smp in range(n):
        bands = [_band(masks[smp], i).reshape(KX, H, NCOL) for i in range(K5)]
        # group 0: i=0 (top, rows h-2 valid h>=2), i=1 (bottom, valid h>=1)
        s2[smp, :KX, 2:, 0] = bands[0][:, 2:]
        s2[smp, KX:, 2:, 0] = bands[1][:, 2:]
        s2[smp, :KX, 1, 0] = bands[1][:, 1]   # h=1 special: i=1 on top half
        # group 1: i=2 (top, always), i=3 (bottom, valid h <= H-2)
        s2[smp, :KX, :, 1] = bands[2]
        s2[smp, KX:, :H - 1, 1] = bands[3][:, :H - 1]
        # group 2: i=4 (top, valid h <= H-3); bottom stays zero
        s2[smp, :KX, :H - 2, 2] = bands[4][:, :H - 2]
    return ft2, s2


def kernel(features: np.ndarray, masks: np.ndarray) -> np.ndarray:
    from concourse.bass_utils import run_bass_kernel_spmd

    if 1 not in _compiled:
        _compiled[1] = _build_program(1)
    nc = _compiled[1]

    ft2, s2 = _prep_inputs(np.asarray(features, dtype=np.float32),
                           np.asarray(masks, dtype=np.float32))
    in_maps = [{"ft2": ft2[i], "s2": s2[i]} for i in range(N_CORES)]
    res = run_bass_kernel_spmd(nc, in_maps, list(range(N_CORES)))
    return np.stack(
        [res.results[i]["out"] for i in range(N_CORES)], axis=0
    ).astype(np.float32)
